# revision 5
# baseline (speedup 1.0000x reference)
"""Trainium2 Bass kernel for the DenoisingModule (non-local attention block).

Math (see reference):
    theta = Wt @ x + bt            [B, 128, HW]
    phi   = Wp @ x + bp            [B, 128, HW]
    f     = theta^T @ phi / 16     [B, HW, HW]
    fh    = softmax(f, axis=0)     (over the BATCH axis - PyTorch legacy dim=0)
    den   = fh @ x^T               [B, C, HW]
    out   = den + (Wc @ den + bc)  = (I + Wc) @ den + bc

Sharding: the softmax couples all 8 batch elements at each (n, m) position,
so batch-parallel would need a 64MB cross-device all-reduce.  Instead we
shard the *n* axis (rows of f / output pixels): each of the 8 cores owns
n in [k*512, (k+1)*512), holds full x, and the softmax is fully local.
No collectives at all; host slices inputs and concatenates outputs.

v2 (this file): bf16 datapath.  x / theta / phi / fexp / softmax arithmetic
are bf16 (rel-err budget is 2e-2; measured ~1e-3).  This
  - halves DMA traffic (xn+xt ~34MB/core),
  - gives DVE the 2x_1p packed mode on the softmax adds/muls,
  - halves SBUF so fexp can double-buffer (bufs=2) -> chunk k+1's exp
    overlaps chunk k's normalize+den,
  - batches the 4 xt tile DMAs per (b,chunk) into one [128,4,256] DMA.
Engine placement: exp + R=exp(-ln S) on ScalarE, softmax tree-adds and
fh muls on DVE (2-byte packed), phi bias-copies on DVE, den PSUM->SBUF
accumulation on the otherwise-idle Pool engine, den kept f32 and the
final (I+Wc) matmul in f32r.  PE: all matmuls 1 cycle/row.

The installed walrus rejects any engine/DMA instruction carrying more
than one semaphore wait ("Too many sync wait commands"), but Tile's
sem-assignment emits up to 4.  _split_excess_waits() legalizes the
scheduled program post-hoc by hoisting excess waits onto single-wait
EventSemaphore instructions inserted just before, on the same engine
queue (applied on the hardware path only; CoreSim runs the pre-split
program).
"""

import sys

import numpy as np

B = 8
C = 256
D = C // 2  # 128
HW = 4096
NCORES = 8
NLOC = HW // NCORES  # 512 n-columns per core
MC = 512  # m-chunk size
NCHUNK = HW // MC  # 8
P = 128

TRACE = False
TRACE_CORES = None
TRACE_DIR = None
LAST = {}

# pool-size knobs (TimelineSim-tuned)
XN_BUFS = 8
XT_BUFS = 13
PHI_BUFS = 2
FEXP_BUFS = 2
PSA_BUFS = 2
PSD_BUFS = 2
SMX_BUFS = 1
OUT_BUFS = 2
POOL_LVL1 = False

_prog = None


def _ensure_path():
    try:
        import concourse  # noqa: F401
    except ImportError:
        for p in ("/opt/trn_rl_repo", "/root/.axon_site/_ro/trn_rl_repo"):
            if p not in sys.path:
                sys.path.insert(0, p)
        import concourse  # noqa: F401


def _build(reps=1):
    from contextlib import ExitStack

    import concourse.bass as bass
    import concourse.tile as tile
    from concourse import mybir

    f32 = mybir.dt.float32
    f32r = mybir.dt.float32r
    bf16 = mybir.dt.bfloat16
    AF = mybir.ActivationFunctionType

    nc = bass.Bass(trn_type="TRN2", target_bir_lowering=False, debug=False)

    xs_h = nc.dram_tensor("xs", [B, 2, P, NLOC], bf16, kind="ExternalInput")
    xn_h = nc.dram_tensor("xn", [B, 2, P, HW], bf16, kind="ExternalInput")
    # xt laid out [B, chunk, s, p, c] so one DMA fetches a whole chunk
    xt_h = nc.dram_tensor("xt", [B, NCHUNK, 4, P, C], bf16, kind="ExternalInput")
    wthT_h = nc.dram_tensor("wthT", [C, D], bf16, kind="ExternalInput")
    wphT_h = nc.dram_tensor("wphT", [C, D], bf16, kind="ExternalInput")
    wcT_h = nc.dram_tensor("wcT", [C, C], f32r, kind="ExternalInput")
    bth_h = nc.dram_tensor("bth", [D, 1], f32, kind="ExternalInput")
    bph_h = nc.dram_tensor("bph", [D, 1], f32, kind="ExternalInput")
    bc_h = nc.dram_tensor("bc", [C, 1], f32, kind="ExternalInput")
    out_h = nc.dram_tensor("out", [B, 2, P, NLOC], f32, kind="ExternalOutput")


    with tile.TileContext(nc) as tc:
        with ExitStack() as ctx:
            consts = ctx.enter_context(tc.tile_pool(name="consts", bufs=1))
            theta_p = ctx.enter_context(tc.tile_pool(name="theta", bufs=1))
            xs_p = ctx.enter_context(tc.tile_pool(name="xsp", bufs=2))
            xn_p = ctx.enter_context(tc.tile_pool(name="xnp", bufs=XN_BUFS))
            xt_p = ctx.enter_context(tc.tile_pool(name="xtp", bufs=XT_BUFS))
            phi_p = ctx.enter_context(tc.tile_pool(name="phip", bufs=PHI_BUFS))
            fexp_p = ctx.enter_context(tc.tile_pool(name="fexpp", bufs=FEXP_BUFS))
            smx_p = ctx.enter_context(tc.tile_pool(name="smxp", bufs=SMX_BUFS))
            den_p = ctx.enter_context(tc.tile_pool(name="denp", bufs=1))
            out_p = ctx.enter_context(tc.tile_pool(name="outp", bufs=OUT_BUFS))
            psA = ctx.enter_context(tc.tile_pool(name="psA", bufs=PSA_BUFS, space="PSUM"))
            psD = ctx.enter_context(tc.tile_pool(name="psD", bufs=PSD_BUFS, space="PSUM"))

            # ---- constants ----
            wth_sb = []
            wph_sb = []
            wc_sb = []
            for ck in range(2):
                t = consts.tile([P, D], bf16, name=f"wth{ck}", tag=f"wth{ck}")
                nc.sync.dma_start(out=t, in_=wthT_h.ap()[ck * P:(ck + 1) * P, :])
                wth_sb.append(t)
                t = consts.tile([P, D], bf16, name=f"wph{ck}", tag=f"wph{ck}")
                nc.sync.dma_start(out=t, in_=wphT_h.ap()[ck * P:(ck + 1) * P, :])
                wph_sb.append(t)
                t = consts.tile([P, C], f32r, name=f"wc{ck}", tag=f"wc{ck}")
                nc.sync.dma_start(out=t, in_=wcT_h.ap()[ck * P:(ck + 1) * P, :])
                wc_sb.append(t)
            bth_sb = consts.tile([D, 1], f32, name="bth", tag="bth")
            nc.sync.dma_start(out=bth_sb, in_=bth_h.ap()[:, :])
            bph_sb = consts.tile([D, 1], f32, name="bph", tag="bph")
            nc.sync.dma_start(out=bph_sb, in_=bph_h.ap()[:, :])
            bc_sb = []
            for dk in range(2):
                t = consts.tile([P, 1], f32, name=f"bc{dk}", tag=f"bc{dk}")
                nc.sync.dma_start(out=t, in_=bc_h.ap()[dk * P:(dk + 1) * P, :])
                bc_sb.append(t)

            def emit_rep(rp):
                # ---- theta_local: [d=128, n=512] per batch, scaled 1/16.
                # The xn DMAs of chunk 0 are interleaved with the xs DMAs so
                # phi(0) isn't queued behind all 16 xs transfers. ----
                theta_sb = []
                xn0_tiles = {}
                for b in range(B):
                    ps = psA.tile([P, 2 * NLOC], f32, name=f"{rp}psth{b}", tag="psA")
                    xst = xs_p.tile([P, 2, NLOC], bf16, name=f"{rp}xs{b}", tag="xs")
                    nc.sync.dma_start(out=xst,
                                      in_=xs_h.ap()[b].transpose([1, 0, 2]))
                    xnt = xn_p.tile([P, 2, MC], bf16, name=f"{rp}xn0_{b}", tag="xn")
                    nc.sync.dma_start(
                        out=xnt,
                        in_=xn_h.ap()[b, :, :, 0:MC].transpose([1, 0, 2]),
                    )
                    xn0_tiles[b] = xnt
                    for ck in range(2):
                        nc.tensor.matmul(
                            ps[:, :NLOC], wth_sb[ck], xst[:, ck, :],
                            start=(ck == 0), stop=(ck == 1),
                        )
                    th = theta_p.tile([D, NLOC], bf16, name=f"{rp}theta{b}", tag=f"theta{b}")
                    nc.scalar.activation(th, ps[:, :NLOC], AF.Identity, bias=bth_sb)
                    theta_sb.append(th)

                # ---- main loop over m-chunks (software-pipelined: den matmuls
                # for chunk k are emitted after f~/exp of chunk k+1, so PE keeps
                # feeding ScalarE's exp while DVE normalizes chunk k) ----
                den_sb = [None] * B
                pend = None  # (mc, fexp, xt_t) awaiting den

                def emit_conv(b):
                    # out = (I + Wc) @ den + bc  (f32r matmul, bias-add on DVE)
                    ot = out_p.tile([P, 2, NLOC], f32, name=f"{rp}out{b}", tag="out")
                    for dk in range(2):
                        ps = psA.tile([P, 2 * NLOC], f32, name=f"{rp}pso{b}_{dk}",
                                      tag="psA")
                        for ct in range(2):
                            nc.tensor.matmul(
                                ps[:, :NLOC],
                                wc_sb[ct][:, dk * P:(dk + 1) * P],
                                den_sb[b][:, ct * NLOC:(ct + 1) * NLOC],
                                start=(ct == 0), stop=(ct == 1),
                            )
                        nc.vector.tensor_scalar_add(ot[:, dk, :], ps[:, :NLOC],
                                                    bc_sb[dk])
                    nc.sync.dma_start(out=out_h.ap()[b].transpose([1, 0, 2]), in_=ot)

                def emit_den_b(mc, fexp, xt_t, b):
                    # Pool can't touch PSUM on this walrus, so the PSUM->SBUF
                    # accumulation stays on DVE.  For the final chunk, the conv
                    # for batch b is emitted right after b's last spill so the
                    # tail pipelines instead of serializing.
                    psd = psD.tile([P, 2 * NLOC], f32, name=f"{rp}psd{mc}_{b}", tag="psD")
                    for ct in range(2):
                        for s in range(4):
                            nc.tensor.matmul(
                                psd[:, ct * NLOC:(ct + 1) * NLOC],
                                xt_t[b][:, s, ct * P:(ct + 1) * P],
                                fexp[b][:, s * NLOC:(s + 1) * NLOC],
                                start=(s == 0), stop=(s == 3),
                            )
                    if mc == 0:
                        dn = den_p.tile([P, 2 * NLOC], f32r, name=f"{rp}den{b}",
                                        tag=f"den{b}")
                        nc.vector.tensor_copy(dn, psd)
                        den_sb[b] = dn
                    else:
                        nc.vector.tensor_add(den_sb[b], den_sb[b], psd)
                    if mc == NCHUNK - 1:
                        emit_conv(b)

                for mc in range(NCHUNK):
                    m0 = mc * MC
                    # phi for this chunk: [d=128, m=512] per batch; batches are
                    # paired into one PSUM tile so the bias-copy (ScalarE) runs
                    # once per pair at [128,1024].  Emitted from emit_phi_pair
                    # inside the b loop below so chunk 0's f~ starts as soon as
                    # its own pair is ready.
                    phi_sb = []

                    def emit_phi_pair(bp, mc=mc, m0=m0):
                        ps = psA.tile([P, 2 * NLOC], f32, name=f"{rp}psph{mc}_{bp}",
                                      tag="psA")
                        for bi in range(2):
                            b = 2 * bp + bi
                            if mc == 0:
                                xnt = xn0_tiles[b]
                            else:
                                xnt = xn_p.tile([P, 2, MC], bf16,
                                                name=f"{rp}xn{mc}_{b}", tag="xn")
                                nc.sync.dma_start(
                                    out=xnt,
                                    in_=xn_h.ap()[b, :, :, m0:m0 + MC]
                                    .transpose([1, 0, 2]),
                                )
                            for ck in range(2):
                                nc.tensor.matmul(
                                    ps[:, bi * MC:(bi + 1) * MC], wph_sb[ck],
                                    xnt[:, ck, :],
                                    start=(ck == 0), stop=(ck == 1),
                                )
                        php = phi_p.tile([D, 2, MC], bf16, name=f"{rp}phi{mc}_{bp}",
                                         tag=f"phi{bp}")
                        nc.scalar.activation(php, ps, AF.Identity, bias=bph_sb)
                        phi_sb.append(php)

                    # xT chunk tiles for the den matmuls (one DMA per (b,chunk))
                    xt_t = []
                    for b in range(B):
                        t = xt_p.tile([P, 4, C], bf16, name=f"{rp}xt{mc}_{b}", tag="xt")
                        nc.sync.dma_start(
                            out=t, in_=xt_h.ap()[b, mc].transpose([1, 0, 2]),
                        )
                        xt_t.append(t)

                    # f~ = theta'^T phi, exp -> fexp [m=128, (s,n) free], bf16.
                    # Per batch b: 4 f~ matmuls, then the den matmuls of the
                    # PREVIOUS chunk for the same b -- PE fills the gaps while
                    # Act's exp stream paces the psA rotation, instead of
                    # stalling on it.  First-level softmax tree adds are
                    # interleaved with the exp stream (early pairs on the
                    # otherwise-idle Pool engine, late pairs on DVE); the
                    # previous chunk's spills run on DVE before this chunk's
                    # muls so DVE has work while Act exps.
                    fexp = []
                    lvl1 = []
                    H = 2 * NLOC
                    for b in range(B):
                        if b % 2 == 0:
                            emit_phi_pair(b // 2)
                        fe = fexp_p.tile([P, 4 * NLOC], bf16, name=f"{rp}fexp{mc}_{b}",
                                         tag=f"fexp{b}")
                        if b % 2 == 1:
                            t = smx_p.tile([P, 4 * NLOC], bf16,
                                           name=f"{rp}s{mc}_{b//2}", tag=f"sl1_{b//2}")
                            lvl1.append(t)
                        for sp in range(2):
                            ps = psA.tile([P, 2 * NLOC], f32,
                                          name=f"{rp}psf{mc}_{b}_{sp}", tag="psA")
                            for si in range(2):
                                s = sp * 2 + si
                                nc.tensor.matmul(
                                    ps[:, si * NLOC:(si + 1) * NLOC],
                                    phi_sb[b // 2][:, b % 2, s * P:(s + 1) * P],
                                    theta_sb[b],
                                    start=True, stop=True,
                                )
                            nc.scalar.activation(
                                fe[:, sp * H:(sp + 1) * H], ps, AF.Exp
                            )
                            # first-level tree add for this half as soon as both
                            # batches of the pair have it (sp block == column
                            # half), so the post-exp serial tail is short.
                            if b % 2 == 1:
                                hs = slice(sp * H, (sp + 1) * H)
                                eng = nc.gpsimd if (POOL_LVL1 and b < 4) else nc.vector
                                eng.tensor_add(lvl1[b // 2][:, hs],
                                               fexp[b - 1][:, hs], fe[:, hs])
                        fexp.append(fe)
                        if pend is not None:
                            emit_den_b(pend[0], pend[1], pend[2], b)

                    # finish softmax tree: S = sum_b fexp[b]; R = 1/S; fh = fexp*R.
                    # The tail is computed in two column-halves so the serial
                    # chain (tree tail -> ln -> exp -> first mul) is half as long
                    # and the muls of half 0 overlap Act's R of half 1.
                    s0123 = smx_p.tile([P, 4 * NLOC], bf16, name=f"{rp}s0123_{mc}",
                                       tag="s0123")
                    S = smx_p.tile([P, 4 * NLOC], bf16, name=f"{rp}S{mc}", tag="S")
                    for h in range(2):
                        hs = slice(h * H, (h + 1) * H)
                        nc.vector.tensor_add(s0123[:, hs], lvl1[0][:, hs],
                                             lvl1[1][:, hs])
                        nc.vector.tensor_add(S[:, hs], lvl1[2][:, hs],
                                             lvl1[3][:, hs])
                        nc.vector.tensor_add(S[:, hs], s0123[:, hs], S[:, hs])
                        # R = 1/S as exp(-ln S): two ScalarE ops (same activation
                        # table set as the main exp).  lnS reuses the dead s0123
                        # tile and R overwrites S -- saves 8KB/partition of SBUF.
                        nc.scalar.activation(s0123[:, hs], S[:, hs], AF.Ln)
                        nc.scalar.activation(S[:, hs], s0123[:, hs], AF.Exp,
                                             scale=-1.0)
                        for b in range(B):
                            nc.vector.tensor_mul(fexp[b][:, hs], fexp[b][:, hs],
                                                 S[:, hs])

                    pend = (mc, fexp, xt_t)
                for b in range(B):
                    emit_den_b(pend[0], pend[1], pend[2], b)


            for _rep in range(reps):
                emit_rep(f"r{_rep}_" if reps > 1 else "")

    return nc


def _split_excess_waits(nc, mybir, cap=1):
    """The installed walrus rejects engine instructions carrying more than
    one semaphore wait (setupSyncWait: "Too many sync wait commands"), but
    Tile's sem-assignment emits up to 4.  Legalize post-hoc: merge same-sem
    waits (max value), keep one on the instruction, and hoist the rest onto
    single-wait EventSemaphore instructions inserted just before, on the
    same engine queue (applies to every opcode incl. DMA pseudo-ops)."""
    n_ev = 0
    for fn in nc.m.functions:
        for blk in fn.blocks:
            insts = blk.instructions
            out = []
            changed = False
            for i in insts:
                si = getattr(i, "sync_info", None)
                waits = list(si.on_wait) if si is not None and si.on_wait else []
                if len(waits) > 1:
                    merged = {}
                    for w in waits:
                        k = w.id
                        if k not in merged or merged[k].wait_value < w.wait_value:
                            merged[k] = w
                    waits = list(merged.values())
                    while len(waits) > cap:
                        w = waits.pop(0)
                        ev = mybir.InstEventSemaphore(
                            name=f"{i.name}-wsplit{n_ev}", engine=i.engine)
                        ev.sync_info = mybir.SyncInfo(on_wait=[w], on_update=[])
                        try:
                            ev.debug = i.debug
                        except Exception:
                            pass
                        out.append(ev)
                        n_ev += 1
                    si.on_wait = waits
                    changed = True
                out.append(i)
            if changed:
                blk.instructions = out
    return n_ev


def _to_bf16(a):
    import ml_dtypes
    return np.ascontiguousarray(a).astype(ml_dtypes.bfloat16)


def _host_prep(x, w_theta, b_theta, w_phi, b_phi, w_conv, b_conv):
    x = np.asarray(x, dtype=np.float32)
    w_theta = np.asarray(w_theta, dtype=np.float32)
    b_theta = np.asarray(b_theta, dtype=np.float32)
    w_phi = np.asarray(w_phi, dtype=np.float32)
    b_phi = np.asarray(b_phi, dtype=np.float32)
    w_conv = np.asarray(w_conv, dtype=np.float32)
    b_conv = np.asarray(b_conv, dtype=np.float32)

    xr = x.reshape(B, C, HW)
    xb = _to_bf16(xr)                                  # [B, C, HW] bf16
    xtr = _to_bf16(xr.transpose(0, 2, 1))              # [B, HW, C] bf16
    xtr = np.ascontiguousarray(xtr).reshape(B, NCHUNK, 4, P, C)
    # 1/sqrt(C) = 1/16: exact power-of-two scale folded into theta
    wthT = _to_bf16((w_theta * (1.0 / 16.0)).T)
    wphT = _to_bf16(w_phi.T)
    wcT = np.ascontiguousarray((np.eye(C, dtype=np.float32) + w_conv).T)
    bth = np.ascontiguousarray((b_theta * (1.0 / 16.0)).reshape(D, 1))
    bph = np.ascontiguousarray(b_phi.reshape(D, 1))
    bc = np.ascontiguousarray(b_conv.reshape(C, 1))

    in_maps = []
    for k in range(NCORES):
        xs_k = np.ascontiguousarray(
            xb[:, :, k * NLOC:(k + 1) * NLOC]).reshape(B, 2, P, NLOC)
        in_maps.append({
            "xs": xs_k, "xn": xb.reshape(B, 2, P, HW), "xt": xtr,
            "wthT": wthT, "wphT": wphT, "wcT": wcT,
            "bth": bth, "bph": bph, "bc": bc,
        })
    return in_maps


def kernel(x, w_theta, b_theta, w_phi, b_phi, w_conv, b_conv):
    global _prog
    _ensure_path()
    from concourse.bass_utils import run_bass_kernel_spmd

    if _prog is None:
        _prog = _build()
        from concourse import mybir
        _split_excess_waits(_prog, mybir)

    in_maps = _host_prep(x, w_theta, b_theta, w_phi, b_phi, w_conv, b_conv)
    extra = {}
    if TRACE_CORES:
        extra["trace_cores"] = TRACE_CORES
    if TRACE_DIR:
        extra["tmpdir"] = TRACE_DIR
    res = run_bass_kernel_spmd(
        _prog, in_maps, list(range(NCORES)), trace=TRACE, **extra,
    )
    LAST["res"] = res

    outf = np.empty((B, C, HW), dtype=np.float32)
    for k in range(NCORES):
        outf[:, :, k * NLOC:(k + 1) * NLOC] = res.results[k]["out"].reshape(B, C, NLOC)
    return outf.reshape(B, C, 64, 64)

